# revision 6
# baseline (speedup 1.0000x reference)
"""Trainium2 Bass kernel for a 2-layer LSTM (H=50) + linear head with
autoregressive future steps. Data-parallel over 8 NeuronCores (batch sharded).

Layout (per core, B_core = 2048 samples):
  - Hidden/gate channels live on SBUF partitions; batch lives on the free dim.
  - Batch is split: samples 0:1024 ("lo") use partitions 0:50, samples
    1024:2048 ("hi") use partitions 64:114 (matmul outputs at col-group 64).
  - Gate PSUM tile [128, 2048] = gates [i|f|o|g] x 512 free each; sigmoid over
    i,f,o is one ACT op spanning 3 banks.
  - Biases are folded into the matmuls via constant-1 rows appended to the
    recurrent state tiles (no separate bias adds anywhere).
  - Elementwise path in fp16 (DVE 2x mode); PSUM accumulation in fp32.
"""

import sys
import os
import numpy as np

for _p in ("/opt/trn_rl_repo", "/root/.axon_site/_ro/trn_rl_repo"):
    if os.path.isdir(_p) and _p not in sys.path:
        sys.path.insert(0, _p)
        break

from contextlib import ExitStack

import concourse.bass as bass
import concourse.mybir as mybir
import concourse.tile as tile
from concourse import bacc
from concourse.bass import ds, ts
from concourse.bass_utils import run_bass_kernel_spmd

FP16 = mybir.dt.float16
FP32 = mybir.dt.float32
AF = mybir.ActivationFunctionType

H = 50
B = 16384
NCORES = 8
BC = B // NCORES          # 2048 samples per core
HALF = 1024               # samples per partition-block (lo/hi)
FREE = 512                # matmul moving free dim (one PSUM bank of fp32)

# gate order in PSUM free dim: i, f, o, g  (i,f,o share sigmoid; g is tanh)
# torch gate blocks in weights: i=0, f=1, g=2, o=3
GATE_SRC = [0, 1, 3, 2]   # our slot G -> torch block index


def _build_nc(T, F):
    TT = T + F
    nc = bacc.Bacc("TRN2", target_bir_lowering=False, debug=False,
                   num_devices=NCORES)

    xT = nc.dram_tensor("xT", [T, BC], FP16, kind="ExternalInput")
    W1 = nc.dram_tensor("W1", [128, 200], FP16, kind="ExternalInput")
    W2A = nc.dram_tensor("W2A", [128, 200], FP16, kind="ExternalInput")
    W2B = nc.dram_tensor("W2B", [128, 200], FP16, kind="ExternalInput")
    WL = nc.dram_tensor("WL", [128, 1], FP16, kind="ExternalInput")
    ONES = nc.dram_tensor("ONES", [1, BC // 2], FP16, kind="ExternalInput")
    yT = nc.dram_tensor("yT", [TT, BC], FP32, kind="ExternalOutput")

    with tile.TileContext(nc) as tc, ExitStack() as ctx:
        const = ctx.enter_context(tc.tile_pool(name="const", bufs=1))
        state = ctx.enter_context(tc.tile_pool(name="state", bufs=1))
        spool = ctx.enter_context(tc.tile_pool(name="spool", bufs=4))
        tpool = ctx.enter_context(tc.tile_pool(name="tpool", bufs=4))
        ypool = ctx.enter_context(tc.tile_pool(name="ypool", bufs=2))
        pg1p = ctx.enter_context(tc.tile_pool(name="pg1", bufs=1, space="PSUM"))
        pg2p = ctx.enter_context(tc.tile_pool(name="pg2", bufs=1, space="PSUM"))

        w1 = const.tile([128, 200], FP16, tag="w1")
        w2a = const.tile([128, 200], FP16, tag="w2a")
        w2b = const.tile([128, 200], FP16, tag="w2b")
        wl = const.tile([128, 1], FP16, tag="wl")
        nc.sync.dma_start(out=w1[:], in_=W1.ap())
        nc.sync.dma_start(out=w2a[:], in_=W2A.ap())
        nc.sync.dma_start(out=w2b[:], in_=W2B.ap())
        nc.sync.dma_start(out=wl[:], in_=WL.ap())

        # state tiles: [h1 | x | 1] and [h2 | 1] per lo/hi block, ping-pong x2
        h1x = [state.tile([128, HALF], FP16, tag=f"h1x{b}", name=f"h1x{b}")
               for b in range(2)]
        h2 = [state.tile([128, HALF], FP16, tag=f"h2{b}", name=f"h2{b}")
              for b in range(2)]
        c1 = state.tile([128, HALF], FP16, tag="c1")
        c2 = state.tile([128, HALF], FP16, tag="c2")

        for b in range(2):
            nc.vector.memset(h1x[b][:], 0.0)
            nc.vector.memset(h2[b][:], 0.0)
            # constant-1 rows (engine ops need 32-aligned partition base; DMA not)
            nc.sync.dma_start(out=h1x[b][51:52, :], in_=ONES.ap())
            nc.sync.dma_start(out=h1x[b][115:116, :], in_=ONES.ap())
            nc.sync.dma_start(out=h2[b][50:51, :], in_=ONES.ap())
            nc.sync.dma_start(out=h2[b][114:115, :], in_=ONES.ap())
        nc.vector.memset(c1[:], 0.0)
        nc.vector.memset(c2[:], 0.0)

        # x for step 0
        nc.sync.dma_start(out=h1x[0][50:51, :], in_=xT.ap()[0:1, 0:HALF])
        nc.sync.dma_start(out=h1x[0][114:115, :], in_=xT.ap()[0:1, HALF:2 * HALF])

        def cell(pg_pool, tag, H1Xc_or_h1, H2c, cst, Hn, fs, is_cell2):
            """Emit one LSTM cell for one free-half. Returns nothing.
            is_cell2=False: rhs = h1x (K=52, single mm per gate position)
            is_cell2=True: rhs = h1 part (K=50) + h2 part (K=51), accumulating."""
            pg = pg_pool.tile([128, 2048], FP32, tag=tag)
            for G in range(4):
                gsl = ts(G, FREE)
                wsl = ts(G, H)
                if not is_cell2:
                    nc.tensor.matmul(pg[0:50, gsl], w1[0:52, wsl],
                                     H1Xc_or_h1[0:52, fs], start=True, stop=True)
                    nc.tensor.matmul(pg[64:114, gsl], w1[64:116, wsl],
                                     H1Xc_or_h1[64:116, fs], start=True, stop=True)
                else:
                    nc.tensor.matmul(pg[0:50, gsl], w2a[0:50, wsl],
                                     H1Xc_or_h1[0:50, fs], start=True, stop=False)
                    nc.tensor.matmul(pg[0:50, gsl], w2b[0:51, wsl],
                                     H2c[0:51, fs], start=False, stop=True)
                    nc.tensor.matmul(pg[64:114, gsl], w2a[64:114, wsl],
                                     H1Xc_or_h1[64:114, fs], start=True, stop=False)
                    nc.tensor.matmul(pg[64:114, gsl], w2b[64:115, wsl],
                                     H2c[64:115, fs], start=False, stop=True)

            s1 = spool.tile([128, 1536], FP16, tag="s" + tag)
            nc.scalar.activation(s1[0:114, :], pg[0:114, 0:1536], AF.Sigmoid)
            tg = spool.tile([128, FREE], FP16, tag="g" + tag)
            nc.scalar.activation(tg[0:114, :], pg[0:114, 1536:2048], AF.Tanh)

            tig = tpool.tile([128, FREE], FP16, tag="i" + tag)
            nc.vector.tensor_mul(tig[0:114, :], s1[0:114, 0:512], tg[0:114, :])
            tfc = tpool.tile([128, FREE], FP16, tag="f" + tag)
            nc.vector.tensor_mul(tfc[0:114, :], s1[0:114, 512:1024], cst[0:114, fs])
            nc.vector.tensor_add(cst[0:114, fs], tig[0:114, :], tfc[0:114, :])
            tch = tpool.tile([128, FREE], FP16, tag="c" + tag)
            nc.scalar.activation(tch[0:114, :], cst[0:114, fs], AF.Tanh)
            nc.vector.tensor_mul(Hn[0:50, fs], s1[0:50, 1024:1536], tch[0:50, :])
            nc.vector.tensor_mul(Hn[64:114, fs], s1[64:114, 1024:1536],
                                 tch[64:114, :])

        for t in range(TT):
            cur, nxt = t % 2, (t + 1) % 2
            H1Xc, H1Xn = h1x[cur], h1x[nxt]
            H2c, H2n = h2[cur], h2[nxt]

            for hf in range(2):
                fs = ds(hf * FREE, FREE)
                cell(pg1p, "pg", H1Xc, None, c1, H1Xn, fs, False)
                cell(pg2p, "qg", H1Xn, H2c, c2, H2n, fs, True)

            # y = Wl @ h2_t + bl for both halves; pgy rotates the pg slot
            pgy = pg2p.tile([128, 2048], FP32, tag="qg")
            for hf in range(2):
                fs = ds(hf * FREE, FREE)
                nc.tensor.matmul(pgy[0:1, fs], wl[0:51, :], H2n[0:51, fs],
                                 start=True, stop=True)
                nc.tensor.matmul(pgy[32:33, fs], wl[64:115, :], H2n[64:115, fs],
                                 start=True, stop=True)
            ysb = ypool.tile([128, HALF], FP32, tag="ysb")
            nc.vector.tensor_copy(ysb[0:33, :], pgy[0:33, 0:HALF])
            nc.sync.dma_start(out=yT.ap()[t:t + 1, 0:HALF], in_=ysb[0:1, :])
            nc.sync.dma_start(out=yT.ap()[t:t + 1, HALF:2 * HALF],
                              in_=ysb[32:33, :])

            # input for step t+1
            if t + 1 < T:
                nc.sync.dma_start(out=H1Xn[50:51, :],
                                  in_=xT.ap()[t + 1:t + 2, 0:HALF])
                nc.sync.dma_start(out=H1Xn[114:115, :],
                                  in_=xT.ap()[t + 1:t + 2, HALF:2 * HALF])
            elif t + 1 < TT:
                # y feedback: fp32->fp16 convert at aligned base, then SBUF DMAs
                ysbh = ypool.tile([128, HALF], FP16, tag="ysbh")
                nc.vector.tensor_copy(ysbh[0:33, :], ysb[0:33, :])
                nc.sync.dma_start(out=H1Xn[50:51, :], in_=ysbh[0:1, :])
                nc.sync.dma_start(out=H1Xn[114:115, :], in_=ysbh[32:33, :])

    nc.compile()
    return nc


def _prep_weights(Wih1, Whh1, bih1, bhh1, Wih2, Whh2, bih2, bhh2, Wl, bl):
    b1 = (bih1 + bhh1).astype(np.float32)
    b2 = (bih2 + bhh2).astype(np.float32)

    W1 = np.zeros((128, 200), np.float32)
    W2A = np.zeros((128, 200), np.float32)
    W2B = np.zeros((128, 200), np.float32)
    WL = np.zeros((128, 1), np.float32)
    for G, src in enumerate(GATE_SRC):
        blk = slice(src * H, (src + 1) * H)
        col = slice(G * H, (G + 1) * H)
        for base in (0, 64):
            W1[base:base + 50, col] = Whh1[blk, :].T
            W1[base + 50, col] = Wih1[blk, 0]
            W1[base + 51, col] = b1[blk]
            W2A[base:base + 50, col] = Wih2[blk, :].T
            W2B[base:base + 50, col] = Whh2[blk, :].T
            W2B[base + 50, col] = b2[blk]
    for base in (0, 64):
        WL[base:base + 50, 0] = Wl[0, :]
        WL[base + 50, 0] = bl[0]
    return (W1.astype(np.float16), W2A.astype(np.float16),
            W2B.astype(np.float16), WL.astype(np.float16))


_NC_CACHE = {}
_last_in_maps = None


def _run(x, Wih1, Whh1, bih1, bhh1, Wih2, Whh2, bih2, bhh2, Wl, bl, future,
         trace=False):
    x = np.asarray(x, np.float32)
    nB, T = x.shape
    F = int(future)
    assert nB == B, (nB, B)

    key = (T, F)
    if key not in _NC_CACHE:
        _NC_CACHE[key] = _build_nc(T, F)
    nc = _NC_CACHE[key]

    W1, W2A, W2B, WLt = _prep_weights(
        np.asarray(Wih1, np.float32), np.asarray(Whh1, np.float32),
        np.asarray(bih1, np.float32), np.asarray(bhh1, np.float32),
        np.asarray(Wih2, np.float32), np.asarray(Whh2, np.float32),
        np.asarray(bih2, np.float32), np.asarray(bhh2, np.float32),
        np.asarray(Wl, np.float32), np.asarray(bl, np.float32))

    in_maps = []
    for c in range(NCORES):
        xc = np.ascontiguousarray(x[c * BC:(c + 1) * BC, :].T).astype(np.float16)
        in_maps.append({"xT": xc, "W1": W1, "W2A": W2A, "W2B": W2B,
                        "WL": WLt, "ONES": np.ones((1, BC // 2), np.float16)})

    global _last_in_maps
    _last_in_maps = in_maps
    res = run_bass_kernel_spmd(nc, in_maps, list(range(NCORES)), trace=trace)
    out = np.empty((B, T + F), np.float32)
    for c in range(NCORES):
        out[c * BC:(c + 1) * BC, :] = res.results[c]["yT"].T
    return out, res


def kernel(**inputs):
    out, _ = _run(**inputs)
    return out


# revision 8
# speedup vs baseline: 449.5275x; 449.5275x over previous
"""Trainium2 Bass kernel for a 2-layer LSTM (H=50) + linear head with
autoregressive future steps. Data-parallel over 8 NeuronCores (batch sharded).

Layout (per core, B_core = 2048 samples):
  - Hidden/gate channels live on SBUF partitions; batch lives on the free dim.
  - Batch is split: samples 0:1024 ("lo") use partitions 0:50, samples
    1024:2048 ("hi") use partitions 64:114 (matmul outputs at col-group 64).
  - Gate PSUM tile [128, 2048] = gates [i|f|o|g] x 512 free each; sigmoid over
    i,f,o is one ACT op spanning 3 banks.
  - Biases are folded into the matmuls via constant-1 rows appended to the
    recurrent state tiles (no separate bias adds anywhere).
  - Elementwise path in fp16 (DVE 2x mode); PSUM accumulation in fp32.
"""

import sys
import os
import numpy as np

for _p in ("/opt/trn_rl_repo", "/root/.axon_site/_ro/trn_rl_repo"):
    if os.path.isdir(_p) and _p not in sys.path:
        sys.path.insert(0, _p)
        break

from contextlib import ExitStack

import concourse.bass as bass
import concourse.mybir as mybir
import concourse.tile as tile
from concourse import bacc
from concourse.bass import ds, ts
from concourse.bass_utils import run_bass_kernel_spmd

FP16 = mybir.dt.float16
FP32 = mybir.dt.float32
AF = mybir.ActivationFunctionType

H = 50
B = 16384
NCORES = 8
BC = B // NCORES          # 2048 samples per core
HALF = 1024               # samples per partition-block (lo/hi)
FREE = 512                # matmul moving free dim (one PSUM bank of fp32)

# gate order in PSUM free dim: i, f, o, g  (i,f,o share sigmoid; g is tanh)
# torch gate blocks in weights: i=0, f=1, g=2, o=3
GATE_SRC = [0, 1, 3, 2]   # our slot G -> torch block index


def _build_nc(T, F, pgy_qg=True, sbufs=4, split_o=False):
    TT = T + F
    nc = bacc.Bacc("TRN2", target_bir_lowering=False, debug=False,
                   num_devices=NCORES)

    xT = nc.dram_tensor("xT", [T, BC], FP16, kind="ExternalInput")
    W1 = nc.dram_tensor("W1", [128, 200], FP16, kind="ExternalInput")
    W2A = nc.dram_tensor("W2A", [128, 200], FP16, kind="ExternalInput")
    W2B = nc.dram_tensor("W2B", [128, 200], FP16, kind="ExternalInput")
    WL = nc.dram_tensor("WL", [128, 1], FP16, kind="ExternalInput")
    ONES = nc.dram_tensor("ONES", [1, BC // 2], FP16, kind="ExternalInput")
    yT = nc.dram_tensor("yT", [TT, BC], FP32, kind="ExternalOutput")

    with tile.TileContext(nc) as tc, ExitStack() as ctx:
        const = ctx.enter_context(tc.tile_pool(name="const", bufs=1))
        state = ctx.enter_context(tc.tile_pool(name="state", bufs=1))
        spool = ctx.enter_context(tc.tile_pool(name="spool", bufs=sbufs))
        tpool = ctx.enter_context(tc.tile_pool(name="tpool", bufs=sbufs))
        ypool = ctx.enter_context(tc.tile_pool(name="ypool", bufs=2))
        pg1p = ctx.enter_context(tc.tile_pool(name="pg1", bufs=1, space="PSUM"))
        pg2p = ctx.enter_context(tc.tile_pool(name="pg2", bufs=1, space="PSUM"))

        w1 = const.tile([128, 200], FP16, tag="w1")
        w2a = const.tile([128, 200], FP16, tag="w2a")
        w2b = const.tile([128, 200], FP16, tag="w2b")
        wl = const.tile([128, 1], FP16, tag="wl")
        nc.sync.dma_start(out=w1[:], in_=W1.ap())
        nc.sync.dma_start(out=w2a[:], in_=W2A.ap())
        nc.sync.dma_start(out=w2b[:], in_=W2B.ap())
        nc.sync.dma_start(out=wl[:], in_=WL.ap())

        # state tiles: [h1 | x | 1] and [h2 | 1] per lo/hi block, ping-pong x2
        h1x = [state.tile([128, HALF], FP16, tag=f"h1x{b}", name=f"h1x{b}")
               for b in range(2)]
        h2 = [state.tile([128, HALF], FP16, tag=f"h2{b}", name=f"h2{b}")
              for b in range(2)]
        c1 = state.tile([128, HALF], FP16, tag="c1")
        c2 = state.tile([128, HALF], FP16, tag="c2")

        for b in range(2):
            nc.vector.memset(h1x[b][:], 0.0)
            nc.vector.memset(h2[b][:], 0.0)
            # constant-1 rows (engine ops need 32-aligned partition base; DMA not)
            nc.sync.dma_start(out=h1x[b][51:52, :], in_=ONES.ap())
            nc.sync.dma_start(out=h1x[b][115:116, :], in_=ONES.ap())
            nc.sync.dma_start(out=h2[b][50:51, :], in_=ONES.ap())
            nc.sync.dma_start(out=h2[b][114:115, :], in_=ONES.ap())
        nc.vector.memset(c1[:], 0.0)
        nc.vector.memset(c2[:], 0.0)

        # x for step 0
        nc.sync.dma_start(out=h1x[0][50:51, :], in_=xT.ap()[0:1, 0:HALF])
        nc.sync.dma_start(out=h1x[0][114:115, :], in_=xT.ap()[0:1, HALF:2 * HALF])

        def cell(pg_pool, tag, H1Xc_or_h1, H2c, cst, Hn, fs, is_cell2):
            """Emit one LSTM cell for one free-half. Returns nothing.
            is_cell2=False: rhs = h1x (K=52, single mm per gate position)
            is_cell2=True: rhs = h1 part (K=50) + h2 part (K=51), accumulating."""
            pg = pg_pool.tile([128, 2048], FP32, tag=tag)
            for G in range(4):
                gsl = ts(G, FREE)
                wsl = ts(G, H)
                if not is_cell2:
                    nc.tensor.matmul(pg[0:50, gsl], w1[0:52, wsl],
                                     H1Xc_or_h1[0:52, fs], start=True, stop=True)
                    nc.tensor.matmul(pg[64:114, gsl], w1[64:116, wsl],
                                     H1Xc_or_h1[64:116, fs], start=True, stop=True)
                else:
                    nc.tensor.matmul(pg[0:50, gsl], w2a[0:50, wsl],
                                     H1Xc_or_h1[0:50, fs], start=True, stop=False)
                    nc.tensor.matmul(pg[0:50, gsl], w2b[0:51, wsl],
                                     H2c[0:51, fs], start=False, stop=True)
                    nc.tensor.matmul(pg[64:114, gsl], w2a[64:114, wsl],
                                     H1Xc_or_h1[64:114, fs], start=True, stop=False)
                    nc.tensor.matmul(pg[64:114, gsl], w2b[64:115, wsl],
                                     H2c[64:115, fs], start=False, stop=True)

            s1 = spool.tile([128, 1536], FP16, tag="s" + tag)
            tg = spool.tile([128, FREE], FP16, tag="g" + tag)
            if split_o:
                nc.scalar.activation(s1[0:114, 0:1024], pg[0:114, 0:1024],
                                     AF.Sigmoid)
                nc.scalar.activation(tg[0:114, :], pg[0:114, 1536:2048],
                                     AF.Tanh)
                nc.scalar.activation(s1[0:114, 1024:1536], pg[0:114, 1024:1536],
                                     AF.Sigmoid)
            else:
                nc.scalar.activation(s1[0:114, :], pg[0:114, 0:1536], AF.Sigmoid)
                nc.scalar.activation(tg[0:114, :], pg[0:114, 1536:2048], AF.Tanh)

            tig = tpool.tile([128, FREE], FP16, tag="i" + tag)
            nc.vector.tensor_mul(tig[0:114, :], s1[0:114, 0:512], tg[0:114, :])
            tfc = tpool.tile([128, FREE], FP16, tag="f" + tag)
            nc.vector.tensor_mul(tfc[0:114, :], s1[0:114, 512:1024], cst[0:114, fs])
            nc.vector.tensor_add(cst[0:114, fs], tig[0:114, :], tfc[0:114, :])
            tch = tpool.tile([128, FREE], FP16, tag="c" + tag)
            nc.scalar.activation(tch[0:114, :], cst[0:114, fs], AF.Tanh)
            nc.vector.tensor_mul(Hn[0:50, fs], s1[0:50, 1024:1536], tch[0:50, :])
            nc.vector.tensor_mul(Hn[64:114, fs], s1[64:114, 1024:1536],
                                 tch[64:114, :])

        for t in range(TT):
            cur, nxt = t % 2, (t + 1) % 2
            H1Xc, H1Xn = h1x[cur], h1x[nxt]
            H2c, H2n = h2[cur], h2[nxt]

            for hf in range(2):
                fs = ds(hf * FREE, FREE)
                cell(pg1p, "pg", H1Xc, None, c1, H1Xn, fs, False)
                cell(pg2p, "qg", H1Xn, H2c, c2, H2n, fs, True)

            # y = Wl @ h2_t + bl for both halves; pgy rotates the pg slot
            if pgy_qg:
                pgy = pg2p.tile([128, 2048], FP32, tag="qg")
            else:
                pgy = pg1p.tile([128, 2048], FP32, tag="pg")
            for hf in range(2):
                fs = ds(hf * FREE, FREE)
                nc.tensor.matmul(pgy[0:1, fs], wl[0:51, :], H2n[0:51, fs],
                                 start=True, stop=True)
                nc.tensor.matmul(pgy[32:33, fs], wl[64:115, :], H2n[64:115, fs],
                                 start=True, stop=True)
            ysb = ypool.tile([128, HALF], FP32, tag="ysb")
            nc.vector.tensor_copy(ysb[0:33, :], pgy[0:33, 0:HALF])
            nc.sync.dma_start(out=yT.ap()[t:t + 1, 0:HALF], in_=ysb[0:1, :])
            nc.sync.dma_start(out=yT.ap()[t:t + 1, HALF:2 * HALF],
                              in_=ysb[32:33, :])

            # input for step t+1
            if t + 1 < T:
                nc.sync.dma_start(out=H1Xn[50:51, :],
                                  in_=xT.ap()[t + 1:t + 2, 0:HALF])
                nc.sync.dma_start(out=H1Xn[114:115, :],
                                  in_=xT.ap()[t + 1:t + 2, HALF:2 * HALF])
            elif t + 1 < TT:
                # y feedback: fp32->fp16 convert at aligned base, then SBUF DMAs
                ysbh = ypool.tile([128, HALF], FP16, tag="ysbh")
                nc.vector.tensor_copy(ysbh[0:33, :], ysb[0:33, :])
                nc.sync.dma_start(out=H1Xn[50:51, :], in_=ysbh[0:1, :])
                nc.sync.dma_start(out=H1Xn[114:115, :], in_=ysbh[32:33, :])

    nc.compile()
    return nc


def _prep_weights(Wih1, Whh1, bih1, bhh1, Wih2, Whh2, bih2, bhh2, Wl, bl):
    b1 = (bih1 + bhh1).astype(np.float32)
    b2 = (bih2 + bhh2).astype(np.float32)

    W1 = np.zeros((128, 200), np.float32)
    W2A = np.zeros((128, 200), np.float32)
    W2B = np.zeros((128, 200), np.float32)
    WL = np.zeros((128, 1), np.float32)
    for G, src in enumerate(GATE_SRC):
        blk = slice(src * H, (src + 1) * H)
        col = slice(G * H, (G + 1) * H)
        for base in (0, 64):
            W1[base:base + 50, col] = Whh1[blk, :].T
            W1[base + 50, col] = Wih1[blk, 0]
            W1[base + 51, col] = b1[blk]
            W2A[base:base + 50, col] = Wih2[blk, :].T
            W2B[base:base + 50, col] = Whh2[blk, :].T
            W2B[base + 50, col] = b2[blk]
    for base in (0, 64):
        WL[base:base + 50, 0] = Wl[0, :]
        WL[base + 50, 0] = bl[0]
    return (W1.astype(np.float16), W2A.astype(np.float16),
            W2B.astype(np.float16), WL.astype(np.float16))


_NC_CACHE = {}
_last_in_maps = None


def _run(x, Wih1, Whh1, bih1, bhh1, Wih2, Whh2, bih2, bhh2, Wl, bl, future,
         trace=False):
    x = np.asarray(x, np.float32)
    nB, T = x.shape
    F = int(future)
    assert nB == B, (nB, B)

    key = (T, F)
    if key not in _NC_CACHE:
        _NC_CACHE[key] = _build_nc(T, F)
    nc = _NC_CACHE[key]

    W1, W2A, W2B, WLt = _prep_weights(
        np.asarray(Wih1, np.float32), np.asarray(Whh1, np.float32),
        np.asarray(bih1, np.float32), np.asarray(bhh1, np.float32),
        np.asarray(Wih2, np.float32), np.asarray(Whh2, np.float32),
        np.asarray(bih2, np.float32), np.asarray(bhh2, np.float32),
        np.asarray(Wl, np.float32), np.asarray(bl, np.float32))

    in_maps = []
    for c in range(NCORES):
        xc = np.ascontiguousarray(x[c * BC:(c + 1) * BC, :].T).astype(np.float16)
        in_maps.append({"xT": xc, "W1": W1, "W2A": W2A, "W2B": W2B,
                        "WL": WLt, "ONES": np.ones((1, BC // 2), np.float16)})

    global _last_in_maps
    _last_in_maps = in_maps
    res = run_bass_kernel_spmd(nc, in_maps, list(range(NCORES)), trace=trace)
    out = np.empty((B, T + F), np.float32)
    for c in range(NCORES):
        out[c * BC:(c + 1) * BC, :] = res.results[c]["yT"].T
    return out, res


def kernel(**inputs):
    out, _ = _run(**inputs)
    return out
